# revision 8
# baseline (speedup 1.0000x reference)
"""Trainium2 Bass kernel: single-head attention + FFN transformer block.

All-bf16 matmuls except U (fp8e4 DoubleRow); S^T layout (math per batch b):
  S^T[k,q] = sum_d K[d,k] Q'[d,q] + kb[k]*qm[q]   (Q' = Q^T, masked+transposed
                                                   ON-CHIP via DVE+XBAR; kb = 0
                                                   valid / -1e30 masked)
  E[k,q]   = exp(S^T/32 - ln16)     ACT, fp8e4 out. Valid rows: softmax numer
                                    scaled by 1/16 (cancels in the ratio);
                                    invalid q: E=1/16 uniform over ALL k ->
                                    att = mean(V), matching the reference.
  rowsum   = ones^T E (PE),  recip via DVE, scattered to [q-part] layout.
  U[q,d]   = E^T_tile V      fp8 DoubleRow (K=256/MM), att = U*recip
  qres     = att + Q         fused on DVE (scalar_tensor_tensor) + row sums
  y        = LN1(qres)       var via bn_stats; rsqrt via DVE Newton
  yT       = XBAR dma transpose per qt (off the PE)
  H^T[o,q] = relu(W1T yT + b1)   bf16
  Z[q,d]   = H^T W2T;   out = LN2(y + Z)  (b2 cancels inside LN2)

Sharding: data-parallel, 4 batches per core on 8 cores. Emission order:
attn(b+1) sits between FFN1(b) and FFN2(b) so S(b+1) grabs the S psum
banks before FFN2(b)'s Zps reuses them, and the LN1/transpose latency of
batch b+1 hides under FFN2(b). PSUM: 4 S banks (reused by FFN2) + 2 U +
2 FFN1 = 8.
"""

import sys

sys.path.insert(0, "/opt/trn_rl_repo")

import numpy as np
import ml_dtypes

import concourse.bass as bass
import concourse.bacc as bacc
import concourse.mybir as mybir
from concourse import tile
from concourse.bass_utils import run_bass_kernel_spmd

B, QTL, KTL, D = 32, 512, 512, 1024
NCORES = 8
BL = B // NCORES
P = 128
NQT = QTL // P   # 4
NKT = KTL // P   # 4
NDT = D // P     # 8
NCH = 2          # 512-wide psum chunks per 1024
EPS = 1e-5
NEG = -1.0e30
LN16 = 2.772588722239781

F32 = mybir.dt.float32
F16 = mybir.dt.float16
BF16 = mybir.dt.bfloat16
F8 = mybir.dt.float8e4
AF = mybir.ActivationFunctionType
ALU = mybir.AluOpType
DR = mybir.MatmulPerfMode.DoubleRow
NP_BF16 = ml_dtypes.bfloat16
NP_F8 = ml_dtypes.float8_e4m3


def _build(apply1: bool, apply2: bool, b1f: float):
    nc = bacc.Bacc(None, target_bir_lowering=False)

    HQK = (NDT // 2) * KTL
    KTAd = nc.dram_tensor("KTAp", [BL, P, HQK], BF16, kind="ExternalInput")
    KTBd = nc.dram_tensor("KTBp", [BL, P, HQK], BF16, kind="ExternalInput")
    Vd = nc.dram_tensor("V8p", [BL, P, NKT * D], F8, kind="ExternalInput")
    Qd = nc.dram_tensor("Qp", [BL, P, NQT * D], BF16, kind="ExternalInput")
    QMd = nc.dram_tensor("QMp", [BL, QTL], BF16, kind="ExternalInput")
    QMCd = nc.dram_tensor("QMCp", [BL, P, NQT], F32, kind="ExternalInput")
    KBd = nc.dram_tensor("KBp", [BL, KTL], BF16, kind="ExternalInput")
    W1Td = nc.dram_tensor("W1Tp", [P, NDT * D], BF16, kind="ExternalInput")
    W2Td = nc.dram_tensor("W2Tp", [P, NDT * D], BF16, kind="ExternalInput")
    if apply1:
        G1d = nc.dram_tensor("G1p", [D], F32, kind="ExternalInput")
        B1d = nc.dram_tensor("B1p", [D], F32, kind="ExternalInput")
    if apply2:
        G2d = nc.dram_tensor("G2p", [D], F32, kind="ExternalInput")
        B2d = nc.dram_tensor("B2p", [D], F32, kind="ExternalInput")
    OUTd = nc.dram_tensor("OUTp", [BL, NQT, P, D], F16, kind="ExternalOutput")

    with tile.TileContext(nc) as tc:
        with (
            tc.tile_pool(name="const", bufs=1) as pc,
            tc.tile_pool(name="wts", bufs=1) as pw,
            tc.tile_pool(name="qkin", bufs=2) as pin,
            tc.tile_pool(name="mid", bufs=2) as pmid,
            tc.tile_pool(name="eh", bufs=1) as peh,
            tc.tile_pool(name="stream", bufs=2) as pst,
            tc.tile_pool(name="small", bufs=2) as psm,
            tc.tile_pool(name="psS", bufs=1, space="PSUM") as psS,
            tc.tile_pool(name="psU", bufs=2, space="PSUM") as psU,
            tc.tile_pool(name="psF", bufs=2, space="PSUM") as psF,
        ):
            # ---- constants / warmup ----
            wz = pc.tile([P, P], BF16)
            nc.vector.memset(wz, 0.0)
            wz512 = pc.tile([P, 512], BF16)
            nc.vector.memset(wz512, 0.0)
            onesb = pc.tile([P, 1], F8)
            nc.vector.memset(onesb, 1.0)
            nln16b = pc.tile([P, 1], F32)
            nc.vector.memset(nln16b, -LN16)
            s32b = pc.tile([P, 1], F32)
            nc.vector.memset(s32b, 1.0 / 32.0)
            epsb = pc.tile([P, 1], F32)
            nc.vector.memset(epsb, EPS)
            b1b = pc.tile([P, 1], F32)
            nc.vector.memset(b1b, b1f)
            if apply1:
                g1t = pc.tile([P, D], F32)
                nc.gpsimd.dma_start(
                    g1t, bass.AP(tensor=G1d, offset=0, ap=[[0, P], [1, D]]))
                b1t = pc.tile([P, D], F32)
                nc.gpsimd.dma_start(
                    b1t, bass.AP(tensor=B1d, offset=0, ap=[[0, P], [1, D]]))
            if apply2:
                g2t = pc.tile([P, D], F32)
                nc.gpsimd.dma_start(
                    g2t, bass.AP(tensor=G2d, offset=0, ap=[[0, P], [1, D]]))
                b2t = pc.tile([P, D], F32)
                nc.gpsimd.dma_start(
                    b2t, bass.AP(tensor=B2d, offset=0, ap=[[0, P], [1, D]]))

            W1Th = [pw.tile([P, NDT // 2, D], BF16, name=f"w1{h}")
                    for h in range(2)]
            W2Th = [pw.tile([P, NDT // 2, D], BF16, name=f"w2{h}")
                    for h in range(2)]

            def load_batch(b):
                # b0's tensors first on every queue: head is HBM-BW bound
                Qf = pin.tile([P, NQT, D], BF16, tag="q", name=f"qf{b}")
                nc.gpsimd.dma_start(Qf.rearrange("p t d -> p (t d)"), Qd[b])
                KTs = pin.tile([P, NDT, KTL], BF16, tag="kt", name=f"kts{b}")
                ktf = KTs.rearrange("p t k -> p (t k)")
                nc.sync.dma_start(ktf[:, :HQK], KTAd[b])
                nc.sync.dma_start(ktf[:, HQK:], KTBd[b])
                V8 = pin.tile([P, NKT // 2, D, 2], F8, tag="v",
                              name=f"v8{b}")
                nc.gpsimd.dma_start(
                    V8.rearrange("p t d i -> p (t d i)"), Vd[b])
                qmt = psm.tile([1, QTL], BF16, tag="qm", bufs=3)
                nc.sync.dma_start(qmt, QMd[b:b + 1, :])
                kbt = psm.tile([1, KTL], BF16, tag="kb", bufs=3)
                nc.sync.dma_start(kbt, KBd[b:b + 1, :])
                qmc = psm.tile([P, NQT], F32, tag="qmc", bufs=3)
                nc.scalar.dma_start(qmc, QMCd[b])
                return dict(Qf=Qf, KTs=KTs, V8=V8, qmt=qmt, kbt=kbt, qmc=qmc)

            ins = {0: load_batch(0)}

            # PE warmup while batch-0 inputs stream in (~4.3us cold: enough
            # to flip HAM to 8/8 right as the first S matmuls arrive)
            wps = psF.tile([P, 512], F32, tag="f", name="warm")
            for _ in range(10):
                nc.tensor.matmul(wps, wz, wz512, start=True, stop=True)

            state = {}

            def ln_scalars(mv, tagp, nqt=NQT, iters=3):
                # r = rsqrt(var+eps) via DVE Newton (no ACT table switch),
                # nm = -mu*r.  mv: [P, NQT, 2] (mean, var).
                var = mv[:, :, 1]
                vh = psm.tile([P, nqt], F32, tag=f"{tagp}vh")
                nc.vector.tensor_scalar(
                    vh, var, EPS, 0.5, op0=ALU.add, op1=ALU.mult)
                r = psm.tile([P, nqt], F32, tag=f"{tagp}r")
                nc.vector.reciprocal(r, var)
                nc.vector.tensor_scalar(
                    r, r, 1.0, 0.5, op0=ALU.add, op1=ALU.mult)
                nc.vector.tensor_scalar_min(r, r, 1.9)
                t = psm.tile([P, nqt], F32, tag=f"{tagp}t")
                for _ in range(iters):
                    nc.vector.tensor_tensor(t, r, r, ALU.mult)
                    nc.vector.tensor_tensor(t, t, vh, ALU.mult)
                    nc.vector.tensor_scalar(
                        t, t, -1.0, 1.5, op0=ALU.mult, op1=ALU.add)
                    nc.vector.tensor_tensor(r, r, t, ALU.mult)
                nm = psm.tile([P, nqt], F32, tag=f"{tagp}nm")
                nc.vector.tensor_tensor(nm, mv[:, :, 0], r, ALU.mult)
                nc.vector.tensor_scalar_mul(nm, nm, -1.0)
                return r, nm

            def pre_attn(b):
                # mask Q by row-validity (per-partition scalar), then XBAR
                # transpose to [d-part, dt, q] layout for the S matmuls
                t = ins[b]
                Qf, qmc = t["Qf"], t["qmc"]
                QMq = pin.tile([P, NQT, D], BF16, tag="qmq", name=f"qmq{b}")
                for qt in range(NQT):
                    nc.vector.tensor_scalar_mul(
                        QMq[:, qt, :], Qf[:, qt, :], qmc[:, qt:qt + 1])
                QTt = pin.tile([P, NDT, QTL], BF16, tag="qtt", name=f"qtt{b}")
                # all XBAR transposes on ONE queue: concurrent transposes on
                # two HWDGE queues race on the crossbar and corrupt blocks
                for qt in range(NQT):
                    nc.scalar.dma_start_transpose(
                        QTt[:, :, qt * P:(qt + 1) * P], QMq[:, qt, :])
                t["QTt"] = QTt

            def attn(b):
                t = ins[b]
                KTs, QTt = t["KTs"], t["QTt"]
                V8, Qf = t["V8"], t["Qf"]
                qmt, kbt = t["qmt"], t["kbt"]

                # --- S^T = K^T-tiles . Q'T (+ kb x qm), 4 psum banks ---
                Sps = [psS.tile([P, QTL], F32, tag=f"s{kt}", name=f"sps{kt}_{b}")
                       for kt in range(NKT)]
                for dt in range(NDT):
                    for kt in range(NKT):
                        nc.tensor.matmul(
                            Sps[kt],
                            KTs[:, dt, kt * P:(kt + 1) * P],
                            QTt[:, dt, :],
                            start=(dt == 0), stop=False)
                for kt in range(NKT):
                    nc.tensor.matmul(
                        Sps[kt], kbt[:, kt * P:(kt + 1) * P], qmt[:, :],
                        start=False, stop=True)

                # --- E = exp(S/32 - ln16), fp8e4 ---
                E = peh.tile([P, NKT, QTL], F8, tag="e", name=f"e{b}")
                for kt in range(NKT):
                    nc.scalar.activation(
                        E[:, kt, :], Sps[kt], AF.Exp,
                        bias=nln16b[:, :], scale=s32b[:, :])

                # --- rowsum^T per q-tile (E_tile^T @ ones), recip ---
                rsps = psU.tile([P, NQT], F32, tag="u", name=f"rsps{b}")
                for qt in range(NQT):
                    for kt in range(NKT):
                        nc.tensor.matmul(
                            rsps[:, qt:qt + 1],
                            E[:, kt, qt * P:(qt + 1) * P],
                            onesb[:, :],
                            start=(kt == 0), stop=(kt == NKT - 1))
                recT = psm.tile([P, NQT], F32, tag="recT")
                nc.vector.reciprocal(recT, rsps)

                # --- U = E^T V (fp8 DoubleRow, K=256/mm) + drain, LN1 stats ---
                mv1 = psm.tile([P, NQT, 2], F32, tag="mv1")
                qres_l = []
                for qt in range(NQT):
                    qres = pst.tile([P, D], F32, tag="qres", bufs=4,
                                    name=f"qres{qt}_{b}")
                    qres_l.append(qres)
                    st1 = psm.tile([P, NCH, 6], F32, tag="st1")
                    for ch in range(NCH):
                        Ups = psU.tile([P, 512], F32, tag="u")
                        for k2 in range(NKT // 2):
                            rhs = V8[:, k2, ch * 512:(ch + 1) * 512, :]
                            nc.tensor.matmul(
                                Ups,
                                E[:, 2 * k2:2 * k2 + 2,
                                  qt * P:(qt + 1) * P],
                                rhs.rearrange("p d i -> p i d"),
                                start=(k2 == 0), stop=(k2 == NKT // 2 - 1),
                                perf_mode=DR)
                        qch = qres[:, ch * 512:(ch + 1) * 512]
                        nc.vector.scalar_tensor_tensor(
                            qch, Ups, recT[:, qt:qt + 1],
                            Qf[:, qt, ch * 512:(ch + 1) * 512],
                            op0=ALU.mult, op1=ALU.add)
                        nc.vector.bn_stats(st1[:, ch, :], qch)
                    nc.vector.bn_aggr(mv1[:, qt, :], st1)

                r1, nm1 = ln_scalars(mv1, "l1")

                # --- y = LN1(qres) bf16; yT via XBAR per qt ---
                y = pmid.tile([P, NQT, D], BF16, tag="y", name=f"y{b}")
                YT4 = pmid.tile([P, NQT, NDT, P], BF16, tag="yt4",
                                name=f"yt4{b}")
                for qt in range(NQT):
                    nc.scalar.activation(
                        y[:, qt, :], qres_l[qt], AF.Identity,
                        bias=nm1[:, qt:qt + 1], scale=r1[:, qt:qt + 1])
                    if apply1:
                        yf = y[:, qt, :]
                        nc.vector.tensor_mul(yf, yf, g1t)
                        nc.vector.tensor_add(yf, yf, b1t)
                    nc.scalar.dma_start_transpose(YT4[:, qt, :, :],
                                                  y[:, qt, :])
                state[b] = dict(y=y, YT4=YT4)

            def ffn1(b):
                YT4 = state[b]["YT4"]
                # --- FFN1: H^T = relu(W1T . yT + b1) ---
                HT = peh.tile([P, NDT, QTL], BF16, tag="ht", name=f"ht{b}")
                for ot in range(NDT):
                    Hps = psF.tile([P, QTL], F32, tag="f")
                    for dt in range(NDT):
                        nc.tensor.matmul(
                            Hps,
                            W1Th[dt // 4][:, dt % 4, ot * P:(ot + 1) * P],
                            YT4[:, :, dt, :],
                            start=(dt == 0), stop=(dt == NDT - 1))
                    nc.scalar.activation(HT[:, ot, :], Hps, AF.Relu,
                                         bias=b1b[:, :])
                state[b]["HT"] = HT

            def ffn2(b):
                y, HT = state[b]["y"], state[b]["HT"]

                # --- FFN2 + LN2 + store ---
                last = b == BL - 1

                def fin2(qt, r2s, nm2, j=0):
                    stg = pst.tile([P, D], F16, tag="stg", bufs=2,
                                   name=f"stg{qt % 2}_{b}")
                    nc.scalar.activation(
                        stg, r2_l[qt], AF.Identity,
                        bias=nm2[:, j:j + 1], scale=r2s[:, j:j + 1])
                    if apply2:
                        nc.vector.tensor_mul(stg, stg, g2t)
                        nc.vector.tensor_add(stg, stg, b2t)
                    nc.sync.dma_start(OUTd[b][qt], stg)

                if not last:
                    mv2 = psm.tile([P, NQT, 2], F32, tag="mv2")
                r2_l = []
                for qt in range(NQT):
                    r2 = pst.tile([P, D], F32, tag="r2", bufs=4,
                                  name=f"r2_{qt}_{b}")
                    r2_l.append(r2)
                    st2 = psm.tile([P, NCH, 6], F32, tag="st2")
                    for ch in range(NCH):
                        Zps = psS.tile([P, 512], F32,
                                       tag=f"s{(qt * NCH + ch) % 4}",
                                       name=f"zps{qt}{ch}_{b}")
                        for ot in range(NDT):
                            nc.tensor.matmul(
                                Zps,
                                HT[:, ot, qt * P:(qt + 1) * P],
                                W2Th[ot // 4][:, ot % 4, ch * 512:(ch + 1) * 512],
                                start=(ot == 0), stop=(ot == NDT - 1))
                        rch = r2[:, ch * 512:(ch + 1) * 512]
                        nc.vector.scalar_tensor_tensor(
                            rch, Zps, 1.0,
                            y[:, qt, ch * 512:(ch + 1) * 512],
                            op0=ALU.mult, op1=ALU.add)
                        nc.vector.bn_stats(st2[:, ch, :], rch)
                    if last:
                        mvq = psm.tile([P, 1, 2], F32, tag=f"mv2q{qt % 2}")
                        nc.vector.bn_aggr(mvq[:, 0, :], st2)
                        r2s, nm2 = ln_scalars(mvq, f"l2q{qt % 2}", nqt=1,
                                              iters=2)
                        fin2(qt, r2s, nm2)
                    else:
                        nc.vector.bn_aggr(mv2[:, qt, :], st2)

                if not last:
                    r2s, nm2 = ln_scalars(mv2, "l2", iters=2)
                    for qt in range(NQT):
                        fin2(qt, r2s, nm2, j=qt)

            assert BL == 4
            hw = NDT * D // 2
            for h in range(2):
                nc.sync.dma_start(
                    W1Th[h].rearrange("p t o -> p (t o)"),
                    W1Td[:, h * hw:(h + 1) * hw])
            ins[1] = load_batch(1)
            for h in range(2):
                nc.sync.dma_start(
                    W2Th[h].rearrange("p t o -> p (t o)"),
                    W2Td[:, h * hw:(h + 1) * hw])
            pre_attn(0)
            attn(0)
            ins[2] = load_batch(2)
            pre_attn(1)
            ffn1(0)
            attn(1)
            ffn2(0)
            ins[3] = load_batch(3)
            pre_attn(2)
            ffn1(1)
            attn(2)
            ffn2(1)
            pre_attn(3)
            ffn1(2)
            attn(3)
            ffn2(2)
            ffn1(3)
            ffn2(3)

    nc.finalize()
    return nc


def _prepare(Q, K, V, Q_lengths, K_lengths, W1, b1, W2, b2,
             ln1_g, ln1_b, ln2_g, ln2_b):
    Q = np.asarray(Q, dtype=np.float32)
    K = np.asarray(K, dtype=np.float32)
    V = np.asarray(V, dtype=np.float32)
    W1 = np.asarray(W1, dtype=np.float32)
    W2 = np.asarray(W2, dtype=np.float32)
    qlen = np.asarray(Q_lengths).astype(np.int64)
    klen = np.asarray(K_lengths).astype(np.int64)
    g1 = np.asarray(ln1_g, dtype=np.float32)
    b1v = np.asarray(ln1_b, dtype=np.float32)
    g2 = np.asarray(ln2_g, dtype=np.float32)
    b2v = np.asarray(ln2_b, dtype=np.float32)
    b1f = float(np.asarray(b1, dtype=np.float32).reshape(-1)[0])
    # b2 cancels exactly inside LN2.

    apply1 = not (np.all(g1 == 1.0) and np.all(b1v == 0.0))
    apply2 = not (np.all(g2 == 1.0) and np.all(b2v == 0.0))

    def tile_rows(x):
        # [B, R, C] -> [B, P, (R/P)*C] in the "(t p)" SBUF tile layout
        Bn, R, C = x.shape
        return np.ascontiguousarray(
            x.reshape(Bn, R // P, P, C).transpose(0, 2, 1, 3).reshape(
                Bn, P, (R // P) * C))

    qmask = (np.arange(QTL)[None, :] < qlen[:, None])
    KT = tile_rows(np.ascontiguousarray(K.transpose(0, 2, 1)).astype(NP_BF16))
    HQK = KT.shape[2] // 2
    KTA = np.ascontiguousarray(KT[:, :, :HQK])
    KTB = np.ascontiguousarray(KT[:, :, HQK:])
    # DoubleRow rhs wants the k-pair adjacent in memory: [B,P,kt2,d,i]
    Vt = V.astype(NP_F8).reshape(B, NKT // 2, 2, P, D)      # [B,kt2,i,p,d]
    V8 = np.ascontiguousarray(
        Vt.transpose(0, 3, 1, 4, 2)).reshape(B, P, NKT * D)  # p,kt2,d,i
    Qb = tile_rows(Q.astype(NP_BF16))
    qmb = qmask.astype(NP_BF16)
    qmc = np.ascontiguousarray(
        qmask.reshape(B, NQT, P).transpose(0, 2, 1)).astype(np.float32)
    kbb = np.where(np.arange(KTL)[None, :] < klen[:, None], 0.0, NEG
                   ).astype(NP_BF16)
    W1T = tile_rows(np.ascontiguousarray(W1.T).astype(NP_BF16)[None])[0]
    W2T = tile_rows(np.ascontiguousarray(W2.T).astype(NP_BF16)[None])[0]

    nc = _build(apply1, apply2, b1f)

    in_maps = []
    for c in range(NCORES):
        s = slice(c * BL, (c + 1) * BL)
        m = {
            "KTAp": KTA[s], "KTBp": KTB[s], "V8p": V8[s], "Qp": Qb[s],
            "QMp": qmb[s], "QMCp": qmc[s], "KBp": kbb[s],
            "W1Tp": W1T, "W2Tp": W2T,
        }
        if apply1:
            m["G1p"] = g1
            m["B1p"] = b1v
        if apply2:
            m["G2p"] = g2
            m["B2p"] = b2v
        in_maps.append(m)

    return nc, in_maps


def kernel(**inputs):
    nc, in_maps = _prepare(**inputs)
    res = run_bass_kernel_spmd(nc, in_maps, list(range(NCORES)))
    out = np.concatenate(
        [res.results[c]["OUTp"].reshape(BL, QTL, D) for c in range(NCORES)],
        axis=0)
    return out.astype(np.float32)
